# revision 22
# baseline (speedup 1.0000x reference)
"""Dilated MHSA block (B=2, N=2048, D=1024, H=16, band +/-16 step 2) on 8 NeuronCores.

Sharding: row-parallel. Each core owns 512 consecutive query rows of one batch
(2 batches x 4 row-blocks = 8 cores) plus a 16-token halo of keys/values on
each side. No collectives; outputs concatenate.

All matmul operands are bf16 (halves weight/x DMA; PE runs 1 cycle/row).
Softmax/norm math stays fp32 on ACT/DVE.

Per-core pipeline:
  A) QKV projection. q,k feature-major ([d, tokens]); v token-major with a
     ones column per head (softmax denominator). QK norm: Square on ACT,
     partition reduction via GPSIMD partition_all_reduce, rsqrt as
     exp(-0.5*ln(ss+eps)) on ACT (one table set), then one fused
     scalar_tensor_tensor: qk_hat = (psum + bias) * inv.
  B) Attention per (head, 256-query half): keys processed in aligned
     128-token tiles; 96 interior queries per tile need only that tile,
     32 boundary queries contract two 32-key chunks. The dilated band mask
     is folded in as a 0/-30 bias tile written to PSUM by an identity
     matmul before the score matmuls accumulate onto it; exp on ACT then
     needs no separate mask op. Softmax denominator comes from the ones
     column of v; reciprocal on DVE, partition_broadcast on GPSIMD,
     normalize on DVE into out_hat.
  C) Output projection per 128-token tile, interleaved per query half.
"""

import sys

sys.path.insert(0, "/opt/trn_rl_repo")

from contextlib import ExitStack

import numpy as np

import concourse.bass as bass
import concourse.tile as tile
from concourse import bacc, bass_isa, mybir
from concourse.bass_utils import run_bass_kernel_spmd

F32 = mybir.dt.float32
BF16 = mybir.dt.bfloat16
AF = mybir.ActivationFunctionType
ALU = mybir.AluOpType

NB, NSEQ, DMODEL = 2, 2048, 1024
NH, DH = 16, 64
HALO = 16          # k * dilation
DILATION = 2
NCORES = 8
ROWS = NSEQ * NB // NCORES   # 512 query rows per core
TLOC = ROWS + 2 * HALO       # 544 local tokens
NTT = 5                      # token tiles: 4x128 + 32
EPS2 = 1e-12                 # guards ln(0) on zero-padded halo tokens
MASKB = -30.0                # additive mask bias; exp(-30) ~ 1e-13


def _emit(ctx, tc, xT, wqkvT, woutT, bqk, bv, bout, biasT, ident, sel2_dram, out):
    nc = tc.nc
    consts = ctx.enter_context(tc.tile_pool(name="consts", bufs=1))
    big = ctx.enter_context(tc.tile_pool(name="big", bufs=1))

    # --- constants (gpsimd SWDGE queue; SP queue stays on the critical path)
    bqk_sb = consts.tile([128, 16], F32)    # [partition within f-tile, f-tile]
    nc.gpsimd.dma_start(
        bqk_sb, bass.AP(tensor=bqk.tensor, offset=bqk.offset, ap=[[1, 128], [128, 16]])
    )
    biasT_sb = consts.tile([128, 2, 3, 256], BF16)  # 0/1 band mask, j-blocks
    nc.gpsimd.dma_start(biasT_sb, biasT)
    eps128 = consts.tile([128, 1], F32)
    nc.vector.memset(eps128, EPS2)
    U32 = mybir.dt.uint32
    ONE = 0x3F800000  # 1.0f bit pattern (valid fp32r: low 12 bits zero)
    F32R = mybir.dt.float32r
    onespair = consts.tile([128, 2], F32R)   # per-head partition-sum weights
    nc.vector.memset(onespair.bitcast(U32), 0)
    nc.vector.memset(onespair[0:64, 0:1].bitcast(U32), ONE)
    nc.vector.memset(onespair[64:128, 1:2].bitcast(U32), ONE)
    sel2 = consts.tile([2, 128], F32R)       # partition broadcast selector
    nc.gpsimd.dma_start(sel2, sel2_dram)
    ones64 = consts.tile([1, 64], F32R)
    nc.vector.memset(ones64.bitcast(U32), ONE)
    bv_sb = consts.tile([128, DMODEL], F32)
    nc.gpsimd.dma_start(
        bv_sb, bass.AP(tensor=bv.tensor, offset=bv.offset, ap=[[0, 128], [1, DMODEL]])
    )
    bout_sb = consts.tile([128, DMODEL], F32)
    wout_sb = big.tile([128, 8, DMODEL], BF16)      # w_out^T, D-major

    qk_sb = big.tile([128, 16, TLOC], BF16)         # q_hat^T / k_hat^T (f-tiles)
    v_aug = big.tile([128, NTT, NH, DH + 1], BF16)  # token-major v + ones col
    nc.vector.memset(v_aug[:, :, :, DH : DH + 1], 1.0)
    out_hat = big.tile([128, 8, ROWS], BF16)        # normalized attn out^T

    # attention-phase SBUF pools are allocated up front so attention emission
    # can overlap phase A's tail (a later pool would wait for space release)
    et_pool = ctx.enter_context(tc.tile_pool(name="et", bufs=2))
    rd_pool = ctx.enter_context(tc.tile_pool(name="rd", bufs=2))
    bcr_pool = ctx.enter_context(tc.tile_pool(name="bcr", bufs=2))
    osb_pool = ctx.enter_context(tc.tile_pool(name="osb", bufs=2))
    P = {"et": et_pool, "rd": rd_pool, "bcr": bcr_pool, "osb": osb_pool}

    def qk_side(hp, side, xT_sb):
        """QKV projection + QK-norm for q f-tile hp (side 0) / k f-tile 8+hp.

        q covers the 512 real query tokens in one sweep; k covers the full
        544-token halo in two 272-token chunks.
        """
        ft = hp + 8 * side
        wt = P["wqk"].tile([128, 8, 128], BF16, tag="wqk")
        f0 = ft * 128
        nc.sync.dma_start(
            wt, wqkvT[:, f0 : f0 + 128].rearrange("(k p) f -> p k f", p=128)
        )
        chunks = [(HALO, ROWS)] if side == 0 else [(0, 272), (272, 272)]
        for off, cw in chunks:
            sl = slice(off, off + cw)
            ps_full = P["qkps"].tile([128, ROWS], F32, tag="qkps", name=f"ps{ft}")
            ps = ps_full[:, 0:cw]
            for k in range(8):
                nc.tensor.matmul(
                    ps, wt[:, k, :], xT_sb[:, k, sl],
                    start=(k == 0), stop=(k == 7),
                )
            raw = qk_sb[:, ft, sl]
            nc.vector.tensor_scalar_add(raw, ps, bqk_sb[:, ft : ft + 1])
            sq_full = P["sq"].tile([128, ROWS], F32R, tag="sq")
            sq = sq_full[:, 0:cw]
            nc.scalar.activation(sq, ps, AF.Square, bias=bqk_sb[:, ft : ft + 1])
            ss_full = P["ssps"].tile([2, ROWS], F32, tag="ssps", name=f"ss{ft}")
            ss = ss_full[:, 0:cw]
            nc.tensor.matmul(ss, onespair, sq, start=True, stop=True)
            # 1/sqrt(ss+eps) = exp(-0.5*ln(ss+eps)): ln/exp/square share one
            # ACT table set (see _restrict_act_tables) -- no table reloads.
            sn_full = P["sn"].tile([2, ROWS], F32, tag="sn")
            sn = sn_full[:, 0:cw]
            nc.scalar.activation(sn, ss, AF.Ln, bias=eps128[0:2, :])
            inv_full = P["inv"].tile([2, ROWS], F32R, tag="inv")
            inv = inv_full[:, 0:cw]
            nc.scalar.activation(inv, sn, AF.Exp, scale=-0.5)
            bc_full = P["bcps"].tile([128, ROWS], F32, tag="bcps", name=f"bc{ft}")
            bc = bc_full[:, 0:cw]
            nc.tensor.matmul(bc, sel2, inv, start=True, stop=True)
            nc.vector.tensor_mul(raw, raw, bc)  # in-place normalize

    def v_chunk(c, xT_sb):
        """V projection for feature chunk c (heads 8c..8c+8) into v_aug."""
        wv = P["wv"].tile([128, 8, 512], BF16, tag="wv")
        nc.sync.dma_start(
            wv,
            wqkvT[:, 2048 + c * 512 : 2048 + (c + 1) * 512].rearrange(
                "(k p) f -> p k f", p=128
            ),
        )
        for tt in range(NTT):
            pt = 128 if tt < 4 else TLOC - 512
            vp = P["vps"].tile([128, 512], F32, tag="vps")
            for k in range(8):
                nc.tensor.matmul(
                    vp[0:pt, :],
                    xT_sb[:, k, tt * 128 : tt * 128 + pt],
                    wv[:, k, :],
                    start=(k == 0), stop=(k == 7),
                )
            nc.vector.tensor_add(
                v_aug[0:pt, tt, c * 8 : (c + 1) * 8, 0:DH],
                vp[0:pt, :].rearrange("p (h d) -> p h d", d=DH),
                bv_sb[0:pt, c * 512 : (c + 1) * 512].rearrange("p (h d) -> p h d", d=DH),
            )

    def attn(h, mu):
        """Banded attention for head h, query half mu (256 queries).

        Per half: key tiles ta=2mu, tb=2mu+1 (and the first 32 keys of
        tc=2mu+2). Packed psum columns, in query order:
          [0:96)    interior of ta   (queries local 256mu+16..+112)
          [96:128)  boundary ta/tb   (queries local 256mu+112..+144)
          [128:224) interior of tb   (queries local 256mu+144..+240)
          [224:256) boundary tb/tc   (queries local 256mu+240..+272)
        Boundary scores: left 32 keys sit on partitions 96:128 (tail of the
        left tile), right 32 keys on partitions 0:32 (head of the right
        tile); partitions 32:96 get only the -30 bias -> exp ~ 0.
        """
        ftq, ftk, pb = h // 2, 8 + h // 2, 64 * (h % 2)
        ta, tb, tc_ = 2 * mu, 2 * mu + 1, 2 * mu + 2
        l0 = 256 * mu  # local token offset of this half's first query is l0+16
        q_int_a = qk_sb[pb : pb + 64, ftq, l0 + 16 : l0 + 112]
        q_bnd_a = qk_sb[pb : pb + 64, ftq, l0 + 112 : l0 + 144]
        q_int_b = qk_sb[pb : pb + 64, ftq, l0 + 144 : l0 + 240]
        q_bnd_b = qk_sb[pb : pb + 64, ftq, l0 + 240 : l0 + 272]
        k_of = lambda t, p0, pn: qk_sb[pb : pb + 64, ftk, 128 * t + p0 : 128 * t + p0 + pn]

        # baseline j-block structure: keys for this 256-query chunk in three
        # blocks of [128, 128, 32] starting at local token 256*mu; mask is a
        # 0/1 multiply on the (bf16, 2x-rate) Pool engine
        q_ap = qk_sb[pb : pb + 64, ftq, HALO + mu * 256 : HALO + mu * 256 + 256]
        sc = P["scps"].tile([128, 3, 256], F32, tag="sc")
        for j in range(3):
            kw = 128 if j < 2 else 32
            kj0 = mu * 256 + j * 128
            nc.tensor.matmul(
                sc[0:kw, j, :],
                qk_sb[pb : pb + 64, ftk, kj0 : kj0 + kw],
                q_ap,
                start=True, stop=True,
            )
        et = P["et"].tile([128, 3, 256], BF16, tag="et")
        nc.scalar.activation(et[:, 0:2, :], sc[:, 0:2, :], AF.Exp)
        nc.scalar.activation(et[0:32, 2, :], sc[0:32, 2, :], AF.Exp)
        nc.gpsimd.tensor_mul(et[:, 0:2, :], et[:, 0:2, :], biasT_sb[:, mu, 0:2, :])
        nc.gpsimd.tensor_mul(et[0:32, 2, :], et[0:32, 2, :], biasT_sb[0:32, mu, 2, :])
        av = P["avps"].tile([DH + 1, 256], F32, tag="av")
        for j in range(3):
            kw = 128 if j < 2 else 32
            nc.tensor.matmul(
                av,
                v_aug[0:kw, mu * 2 + j, h, :],
                et[0:kw, j, :],
                start=(j == 0), stop=(j == 2),
            )

        rd = P["rd"].tile([1, 256], mybir.dt.float32r, tag="rd")
        with nc.allow_low_precision(reason="fp32r feeds PE"):
            nc.vector.reciprocal(rd, av[DH : DH + 1, :])
        bcr = P["bcrps"].tile([64, 256], F32, tag="bcr")
        nc.tensor.matmul(bcr, ones64, rd, start=True, stop=True)
        avs = P["bcr"].tile([64, 256], F32, tag="avs")
        nc.vector.tensor_copy(avs, av[0:DH, :])
        nc.vector.tensor_mul(
            out_hat[pb : pb + 64, h // 2, 256 * mu : 256 * mu + 256],
            avs, bcr,
        )

    def proj(mu, ts_):
        """Output projection for one 128-row tile of query half mu."""
        row0 = mu * 256 + ts_ * 128
        for ec in range(2):
            po = P["pops"].tile([128, 512], F32, tag="po")
            for ph in range(8):
                nc.tensor.matmul(
                    po,
                    out_hat[:, ph, row0 : row0 + 128],
                    wout_sb[:, ph, ec * 512 : (ec + 1) * 512],
                    start=(ph == 0), stop=(ph == 7),
                )
            osb = P["osb"].tile([128, 512], F32, tag="osb")
            nc.vector.tensor_add(osb, po, bout_sb[:, ec * 512 : (ec + 1) * 512])
            nc.sync.dma_start(out[row0 : row0 + 128, ec * 512 : (ec + 1) * 512], osb)

    # ---- Phase A: QKV + norm (scoped SBUF + PSUM pools) ---------------
    with (
        tc.tile_pool(name="xtp", bufs=1) as _xt,
        tc.tile_pool(name="wqk", bufs=3) as _wqk,
        tc.tile_pool(name="wv", bufs=2) as _wv,
        tc.tile_pool(name="sq", bufs=2) as _sq,
        tc.tile_pool(name="sn", bufs=2) as _sn,
        tc.tile_pool(name="inv", bufs=2) as _inv,
        tc.tile_pool(name="qkps", bufs=3, space="PSUM") as _qk,
        tc.tile_pool(name="ssps", bufs=1, space="PSUM") as _ss,
        tc.tile_pool(name="bcps", bufs=1, space="PSUM") as _bc,
        tc.tile_pool(name="vps", bufs=2, space="PSUM") as _vp,
    ):
        P.update(wqk=_wqk, wv=_wv, sq=_sq, sn=_sn, inv=_inv,
                 qkps=_qk, ssps=_ss, bcps=_bc, vps=_vp)
        xT_sb = _xt.tile([128, 8, TLOC], BF16)       # x^T, d_in-major
        # per-k-chunk DMAs so the first matmuls can start before the full
        # x transfer lands
        for k in range(8):
            nc.sync.dma_start(xT_sb[:, k, :], xT[128 * k : 128 * (k + 1), :])
        # PE warm-up: dummy matmul chain on memset constants during the
        # input-DMA lead-in, so the HAM clock gate (and the cost model's
        # p-state ramp) reaches full rate before the first real matmul.
        warm_lhs = consts.tile([128, 2], BF16)
        nc.vector.memset(warm_lhs, 1.0)
        warm_rhs = consts.tile([128, 272], BF16)
        nc.vector.memset(warm_rhs, 1.0)
        warm_ps = P["qkps"].tile([2, 272], F32, tag="qkps", name="warmps")
        for _ in range(16):
            nc.tensor.matmul(warm_ps, warm_lhs, warm_rhs, start=True, stop=True)
        for hp in range(8):
            qk_side(hp, 0, xT_sb)
            qk_side(hp, 1, xT_sb)
            if hp == 3:
                v_chunk(0, xT_sb)
            if hp == 7:
                v_chunk(1, xT_sb)

    # w_out / b_out loads: SP runs them during the attention phase
    nc.sync.dma_start(wout_sb, woutT.rearrange("(k p) e -> p k e", p=128))
    nc.sync.dma_start(
        bout_sb,
        bass.AP(tensor=bout.tensor, offset=bout.offset, ap=[[0, 128], [1, DMODEL]]),
    )

    # ---- Phases B+C: attention + projection, interleaved per half -----
    with (
        tc.tile_pool(name="scps", bufs=2, space="PSUM") as _sc,
        tc.tile_pool(name="avps", bufs=1, space="PSUM") as _av,
        tc.tile_pool(name="bcrps", bufs=1, space="PSUM") as _bcr,
        tc.tile_pool(name="pops", bufs=2, space="PSUM") as _po,
    ):
        P.update(scps=_sc, avps=_av, bcrps=_bcr, pops=_po)
        for mu in range(2):
            for h in range(NH):
                attn(h, mu)
            for ts_ in range(2):
                proj(mu, ts_)


def _restrict_act_tables():
    """Restrict the ACT table registry to natural_log_exp_and_others, which
    holds every activation this kernel uses (ln/exp/identity/square/copy).
    The default chooser pairs Ln and Exp with different sets, forcing a
    ~1.3us table reload on every ln<->exp alternation."""
    import concourse.hw_specs as hw_specs
    import concourse.bass_interp as bass_interp

    if getattr(_restrict_act_tables, "done", False):
        return
    orig = hw_specs.get_activation_tables

    def only_lnexp(arch):
        # Keep the full set list (set ids index act_info.json, which walrus
        # also reads), but make natural_log_exp_and_others the only set that
        # offers Ln or Exp so the load-placement pass picks it for both.
        t = orig(arch)
        ln = mybir.ActivationFunctionType.Ln
        ex = mybir.ActivationFunctionType.Exp
        out = {}
        for name, funcs in t.items():
            if name != "natural_log_exp_and_others":
                funcs = funcs - {ln, ex}
            out[name] = funcs
        return out

    hw_specs.get_activation_tables = only_lnexp
    bacc.get_activation_tables = only_lnexp
    bass_interp.get_activation_tables = only_lnexp
    _restrict_act_tables.done = True


def build_nc():
    _restrict_act_tables()
    nc = bacc.Bacc(
        "TRN2", target_bir_lowering=False, debug=False, num_devices=NCORES
    )
    xT = nc.dram_tensor("xT", [DMODEL, TLOC], BF16, kind="ExternalInput").ap()
    wqkvT = nc.dram_tensor("wqkvT", [DMODEL, 3 * DMODEL], BF16, kind="ExternalInput").ap()
    woutT = nc.dram_tensor("woutT", [DMODEL, DMODEL], BF16, kind="ExternalInput").ap()
    bqk = nc.dram_tensor("bqk", [2 * DMODEL], F32, kind="ExternalInput").ap()
    bv = nc.dram_tensor("bv", [DMODEL], F32, kind="ExternalInput").ap()
    bout = nc.dram_tensor("bout", [DMODEL], F32, kind="ExternalInput").ap()
    biasT = nc.dram_tensor("biasT", [128, 2, 3, 256], BF16, kind="ExternalInput").ap()
    ident = nc.dram_tensor("ident", [128, 128], BF16, kind="ExternalInput").ap()
    sel2 = nc.dram_tensor("sel2", [2, 128], mybir.dt.float32r, kind="ExternalInput").ap()
    out = nc.dram_tensor("out", [ROWS, DMODEL], F32, kind="ExternalOutput").ap()
    with tile.TileContext(nc) as tc, ExitStack() as ctx:
        _emit(ctx, tc, xT, wqkvT, woutT, bqk, bv, bout, biasT, ident, sel2, out)
    nc.compile()
    return nc


_CACHE = {}


def _get_nc():
    if "nc" not in _CACHE:
        _CACHE["nc"] = build_nc()
    return _CACHE["nc"]


def _core_biasT(n0):
    """0/1 band-mask tile [128, 2, 3, 256] for the core at query offset n0."""
    m = np.zeros((128, 2, 3, 256), np.float32)
    p = np.arange(128)[:, None]
    qi = np.arange(256)[None, :]
    for qc in range(2):
        for j in range(3):
            jg = n0 - HALO + qc * 256 + j * 128 + p  # global key index
            ig = n0 + qc * 256 + qi                  # global query index
            d = ig - jg
            ok = (np.abs(d) <= HALO) & (d % DILATION == 0) & (jg >= 0) & (jg < NSEQ)
            m[:, qc, j, :] = ok
    return m


def _prep_in_maps(x, w_qkv, b_qkv, w_out, b_out):
    import ml_dtypes

    def wcast(a):
        return np.ascontiguousarray(np.asarray(a, np.float32)).astype(ml_dtypes.bfloat16)

    x = np.asarray(x, np.float32)
    wqkvT = wcast(np.asarray(w_qkv, np.float32).T)
    woutT = wcast(np.asarray(w_out, np.float32).T)
    bqk = np.ascontiguousarray(np.asarray(b_qkv, np.float32)[: 2 * DMODEL])
    bv = np.ascontiguousarray(np.asarray(b_qkv, np.float32)[2 * DMODEL :])
    bout = np.ascontiguousarray(np.asarray(b_out, np.float32))
    ident = np.eye(128, dtype=np.float32).astype(ml_dtypes.bfloat16)
    sel2 = np.zeros((2, 128), np.float32)
    sel2[0, 0:64] = 1.0
    sel2[1, 64:128] = 1.0
    in_maps = []
    for c in range(NCORES):
        b, n0 = c // 4, (c % 4) * ROWS
        lo, hi = n0 - HALO, n0 + ROWS + HALO
        xs = np.zeros((TLOC, DMODEL), np.float32)
        src_lo, src_hi = max(lo, 0), min(hi, NSEQ)
        xs[src_lo - lo : src_hi - lo] = x[b, src_lo:src_hi]
        in_maps.append(
            {
                "xT": wcast(xs.T),
                "wqkvT": wqkvT,
                "woutT": woutT,
                "bqk": bqk,
                "bv": bv,
                "bout": bout,
                "biasT": _core_biasT(n0).astype(ml_dtypes.bfloat16),
                "ident": ident,
                "sel2": sel2,
            }
        )
    return in_maps


def run(inputs, trace=False):
    """Returns (full_output, BassKernelResults)."""
    nc = _get_nc()
    in_maps = _prep_in_maps(**inputs)
    res = run_bass_kernel_spmd(nc, in_maps, list(range(NCORES)), trace=trace)
    out = np.empty((NB, NSEQ, DMODEL), np.float32)
    for c in range(NCORES):
        b, n0 = c // 4, (c % 4) * ROWS
        out[b, n0 : n0 + ROWS] = res.results[c]["out"]
    return out, res


def kernel(x, w_qkv, b_qkv, w_out, b_out):
    out, _ = run(
        dict(x=x, w_qkv=w_qkv, b_qkv=b_qkv, w_out=w_out, b_out=b_out), trace=False
    )
    return out
